# revision 6
# baseline (speedup 1.0000x reference)
"""DoubleAttention TRN2 Bass kernel — fp8 DoubleRow + double re-association.

Full inputs in, full outputs out. Data-parallel over batch: B=32 split as
4 batches per core across 8 NeuronCores; weights replicated.

Reference math (C = Cout = dn = 512, N = H*W = 1024):
  A   = wA @ x + bA            [C, N]
  smB = softmax(wB @ x, n)     (bB drops: exp(bB[d]) cancels in row norm)
  smV = softmax(wV @ x, n)     (bV drops)
  G   = A @ smB^T              [C, C]
  out = wR @ (G @ smV) + bR    [C, N]

Re-associated so the spatial-N matmuls touch only x/EB/EV and all
[512,512]-sized products chain cheaply:
  EB  = exp(wB x)  (unnormalized), sB[d] = sum_n EB
  EV  = exp(wV x),                 sV[d] = sum_n EV
  T   = x @ EB^T                       [C, dn]   (K = N contraction)
  G1  = wA @ T                         [C, dn]
  M2T[d,o] = sum_c G1[c,d] wR[o,c]     [dn, C]
  m2s[d,o] = M2T[d,o]/(sB[d] sV[d]) + (wR bA)[o]/sV[d]
  out[o,n] = sum_d m2s[d,o] EV[d,n] + bR[o]
(uses A@smB^T = wA(x smB^T) + bA 1^T and 1^T smV = colsum(smV))

Precision plan (gate is 2e-2 absmax-relative; this lands ~5.6e-3):
  - all big matmuls fp8e4 DoubleRow (K=256/instr, 2x PE throughput)
  - x shipped pre-quantized AND pre-transposed from host; the T matmul
    uses a hi+lo residual split of x^T (unscaled lo residual, so hi and
    lo accumulate into the same PSUM group)
  - wA, wR: hi+lo fp8 splits (x64 scale); wB, wV: single fp8 (x64)
  - exp / softmax stats and all rescales are exact fp32 on small tensors
"""

import numpy as np

B, C, N = 32, 512, 1024  # batch, channels, spatial (32*32)
H = W = 32
NCORES = 8
BPC = B // NCORES   # batches per core
KT = C // 128       # 4 channel tiles
NT = N // 128       # 8 n tiles
NS = N // 512       # 2 n free-dim spans

WS = 64.0           # weight quantization scale
SZ = 2.0 ** 17      # m2s quantization scale

_CACHE = {}


def _build_nc():
    import concourse.bacc as bacc
    import concourse.mybir as mybir
    import concourse.tile as tile

    F32 = mybir.dt.float32
    F32R = mybir.dt.float32r
    F16 = mybir.dt.float16
    F8 = mybir.dt.float8e4
    AF = mybir.ActivationFunctionType
    DR = mybir.MatmulPerfMode.DoubleRow

    nc = bacc.Bacc("TRN2", target_bir_lowering=False, debug=False,
                   num_devices=NCORES)
    xq_d = nc.dram_tensor("xq", [BPC, KT, 128, N], F8, kind="ExternalInput").ap()
    xth_d = nc.dram_tensor("xth", [BPC, NT, 128, C], F8, kind="ExternalInput").ap()
    xtl_d = nc.dram_tensor("xtl", [BPC, NT, 128, C], F8, kind="ExternalInput").ap()
    wbt_d = nc.dram_tensor("wbt", [KT, 128, C], F8, kind="ExternalInput").ap()
    wvt_d = nc.dram_tensor("wvt", [KT, 128, C], F8, kind="ExternalInput").ap()
    wath_d = nc.dram_tensor("wath", [KT, 128, C], F8, kind="ExternalInput").ap()
    watl_d = nc.dram_tensor("watl", [KT, 128, C], F8, kind="ExternalInput").ap()
    wrth_d = nc.dram_tensor("wrth", [KT, 128, C], F8, kind="ExternalInput").ap()
    wrtl_d = nc.dram_tensor("wrtl", [KT, 128, C], F8, kind="ExternalInput").ap()
    qbs_d = nc.dram_tensor("qbs", [128, C], F32, kind="ExternalInput").ap()
    br_d = nc.dram_tensor("br", [128, KT], F32, kind="ExternalInput").ap()
    ones8_d = nc.dram_tensor("ones8", [128, 2, 128], F8, kind="ExternalInput").ap()
    ones_d = nc.dram_tensor("ones", [128, 128], F32R, kind="ExternalInput").ap()
    o_d = nc.dram_tensor("o", [BPC, C, N], F16, kind="ExternalOutput").ap()

    with tile.TileContext(nc) as tc:
        with tc.tile_pool(name="wp", bufs=1) as wp, \
             tc.tile_pool(name="xp", bufs=2) as xp, \
             tc.tile_pool(name="ip", bufs=1) as ip, \
             tc.tile_pool(name="op", bufs=2) as op_, \
             tc.tile_pool(name="sp", bufs=2) as sp, \
             tc.tile_pool(name="pp", bufs=8, space="PSUM") as pp:

            wbt = wp.tile([128, KT, C], F8, tag="wbt")
            wvt = wp.tile([128, KT, C], F8, tag="wvt")
            wath = wp.tile([128, KT, C], F8, tag="wath")
            watl = wp.tile([128, KT, C], F8, tag="watl")
            wrth = wp.tile([128, KT, C], F8, tag="wrth")
            wrtl = wp.tile([128, KT, C], F8, tag="wrtl")
            qbs = wp.tile([128, C], F32, tag="qbs")
            br = wp.tile([128, KT], F32, tag="br")
            ones8 = wp.tile([128, 2, 128], F8, tag="ones8")
            ones = wp.tile([128, 128], F32R, tag="ones")
            xq0 = xp.tile([128, KT, N], F8, tag="xq")
            xth0 = xp.tile([128, NT, C], F8, tag="xth")
            xtl0 = xp.tile([128, NT, C], F8, tag="xtl")

            # Warm the PE HAM clock gate during the DMA head: slow fp32
            # matmuls on a memset tile keep the array busy through the
            # p-state ramp window and finish before the real stream.
            garb = wp.tile([128, 512], F32, tag="garb")
            nc.gpsimd.memset(garb[:], 1.0)
            psw = pp.tile([128, 512], F32, tag="mm")
            for _ in range(2):
                nc.tensor.matmul(psw[:], garb[:, 0:128], garb[:],
                                 start=True, stop=True)

            # DMA priority for batch 0, coalesced (each dma_start costs
            # ~600ns of Sync issue time; transfers stripe over 16 DMA
            # engines regardless of count): B needs xq n-half 0 + wbt,
            # then V needs wvt + half 1, then T needs xth/xtl, ...
            nc.sync.dma_start(xq0[:, :, 0:512],
                              xq_d[0][:, :, 0:512].rearrange("k p n -> p k n"))
            nc.sync.dma_start(wbt[:], wbt_d.rearrange("k p c -> p k c"))
            nc.sync.dma_start(wvt[:], wvt_d.rearrange("k p c -> p k c"))
            nc.sync.dma_start(xq0[:, :, 512:1024],
                              xq_d[0][:, :, 512:1024].rearrange(
                                  "k p n -> p k n"))
            nc.sync.dma_start(ones8[:], ones8_d[:])
            nc.sync.dma_start(xth0[:], xth_d[0].rearrange("t p c -> p t c"))
            nc.sync.dma_start(xtl0[:], xtl_d[0].rearrange("t p c -> p t c"))
            nc.sync.dma_start(wath[:], wath_d.rearrange("k p c -> p k c"))
            nc.sync.dma_start(watl[:], watl_d.rearrange("k p c -> p k c"))
            nc.sync.dma_start(ones[:], ones_d[:])
            nc.sync.dma_start(wrth[:], wrth_d.rearrange("k p c -> p k c"))
            nc.sync.dma_start(wrtl[:], wrtl_d.rearrange("k p c -> p k c"))
            nc.sync.dma_start(qbs[:], qbs_d[:])
            nc.sync.dma_start(br[:], br_d[:])

            def emit_front(b, xq, xth, xtl):
                """Phases B, V, T, sB, G1, psc, M2 for batch b; returns the
                tiles phase Z needs (m2s, ev, os staging)."""
                ebt = ip.tile([128, NT, C], F8, tag="ebt")
                ev = ip.tile([128, KT, N], F8, tag="ev")
                tq = ip.tile([128, KT, C], F8, tag="tq")
                g1q = ip.tile([128, KT, C], F8, tag="g1q")
                m2s = ip.tile([128, KT, C], F8, tag="m2s")
                os_ = op_.tile([128, KT, N], F16, tag="os")
                av = sp.tile([128, KT, NS], F32, tag="av")
                svc = sp.tile([128, KT], F32, tag="svc")
                rsv = sp.tile([128, KT], F32, tag="rsv")
                prod = sp.tile([128, KT], F32, tag="prod")
                prodp = sp.tile([128, KT], F32, tag="prodp")
                rscs = sp.tile([128, KT], F32, tag="rscs")
                sbc = sp.tile([128, KT], F32, tag="sbc")
                sbr = sp.tile([1, C], F32R, tag="sbr")

                # Phase B: EBT[n,d] = exp(Bm^T) via psb = 64*Bm^T
                for nt in range(NT):
                    nsl = slice(nt * 128, (nt + 1) * 128)
                    psb = pp.tile([128, 512], F32, tag="mm")
                    for kp in range(2):
                        ksl = slice(2 * kp, 2 * kp + 2)
                        nc.tensor.matmul(psb[:], xq[:, ksl, nsl],
                                         wbt[:, ksl, :], start=(kp == 0),
                                         stop=(kp == 1), perf_mode=DR)
                    nc.scalar.activation(ebt[:, nt, :], psb[:], AF.Exp,
                                         scale=1.0 / WS)

                # Interleave point: the previous batch's Z phase lands here
                # (emitted by the caller) to hide its m2s-evac latency under
                # this batch's B matmuls.
                yield (m2s, ev, os_)

                # Phase V: EV[d,n] = exp(Vm) + per-row expsums (accum)
                for dt in range(KT):
                    dsl = slice(dt * 128, (dt + 1) * 128)
                    for h in range(NS):
                        hsl = slice(h * 512, (h + 1) * 512)
                        psv = pp.tile([128, 512], F32, tag="mm")
                        for kp in range(2):
                            ksl = slice(2 * kp, 2 * kp + 2)
                            nc.tensor.matmul(psv[:], wvt[:, ksl, dsl],
                                             xq[:, ksl, hsl],
                                             start=(kp == 0),
                                             stop=(kp == 1), perf_mode=DR)
                        nc.scalar.activation(ev[:, dt, hsl], psv[:], AF.Exp,
                                             scale=1.0 / WS,
                                             accum_out=av[:, dt, h:h + 1])
                nc.vector.tensor_add(svc[:], av[:, :, 0], av[:, :, 1])
                nc.vector.reciprocal(rsv[:], svc[:])

                # Phase T: T[c,d] = sum_n xT_eff[n,c] EBT[n,d]; hi and
                # unscaled-lo x splits accumulate into one PSUM group.
                for ct in range(KT):
                    csl = slice(ct * 128, (ct + 1) * 128)
                    pst = pp.tile([128, 512], F32, tag="mm")
                    for xpart, first in ((xth, True), (xtl, False)):
                        for np_ in range(NT // 2):
                            nsl = slice(2 * np_, 2 * np_ + 2)
                            nc.tensor.matmul(
                                pst[:], xpart[:, nsl, csl], ebt[:, nsl, :],
                                start=(first and np_ == 0),
                                stop=((not first) and np_ == NT // 2 - 1),
                                perf_mode=DR)
                    nc.vector.tensor_scalar_mul(tq[:, ct, :], pst[:], 0.125)

                # sB[d] = sum_n EBT[n,d] via ones-matmul over all n tiles
                pss = pp.tile([128, 512], F32, tag="mm")
                for np_ in range(NT // 2):
                    nsl = slice(2 * np_, 2 * np_ + 2)
                    nc.tensor.matmul(pss[:], ones8[:], ebt[:, nsl, :],
                                     start=(np_ == 0),
                                     stop=(np_ == NT // 2 - 1),
                                     perf_mode=DR)
                nc.vector.tensor_copy(sbr[:], pss[0:1, :])

                # Phase G1: psg1[o,d] = 8 * (wA @ T)[o,d]
                for ot in range(KT):
                    osl = slice(ot * 128, (ot + 1) * 128)
                    psg1 = pp.tile([128, 512], F32, tag="mm")
                    mms = [(wath, 0), (wath, 1), (watl, 0), (watl, 1)]
                    for i, (wpart, kp) in enumerate(mms):
                        ksl = slice(2 * kp, 2 * kp + 2)
                        nc.tensor.matmul(psg1[:], wpart[:, ksl, osl],
                                         tq[:, ksl, :], start=(i == 0),
                                         stop=(i == 3), perf_mode=DR)
                    nc.vector.tensor_scalar_mul(g1q[:, ot, :], psg1[:],
                                                1.0 / 32.0)

                # sB row -> per-partition columns via K=1 transpose mms
                psc = pp.tile([128, KT, 2], F32, tag="mm")
                for dtc in range(KT):
                    nc.tensor.matmul(psc[:, dtc, :],
                                     sbr[0:1, dtc * 128:(dtc + 1) * 128],
                                     ones[0:1, 0:2], start=True, stop=True)
                nc.vector.tensor_copy(sbc[:], psc[:, :, 0])
                nc.vector.tensor_mul(prod[:], sbc[:], svc[:])
                nc.vector.tensor_scalar_mul(prodp[:], prod[:], 16.0 / SZ)
                nc.vector.reciprocal(rscs[:], prodp[:])

                # Phase M2: psm[d,o] = 16 * M2T; evac folds softmax
                # normalization + the wR@bA rank-1 bias, into fp8 at SZ.
                for dt in range(KT):
                    dsl = slice(dt * 128, (dt + 1) * 128)
                    psm = pp.tile([128, 512], F32, tag="mm")
                    mms = [(wrth, 0), (wrth, 1), (wrtl, 0), (wrtl, 1)]
                    for i, (wpart, kp) in enumerate(mms):
                        ksl = slice(2 * kp, 2 * kp + 2)
                        nc.tensor.matmul(psm[:], g1q[:, ksl, dsl],
                                         wpart[:, ksl, :], start=(i == 0),
                                         stop=(i == 3), perf_mode=DR)
                    gta = sp.tile([128, C], F32, tag="gta")
                    nc.scalar.mul(gta[:], psm[:], rscs[:, dt:dt + 1])
                    tmpb = sp.tile([128, C], F32, tag="tmpb")
                    nc.vector.tensor_scalar_mul(tmpb[:], qbs[:],
                                                rsv[:, dt:dt + 1])
                    nc.vector.tensor_add(m2s[:, dt, :], gta[:], tmpb[:])

            def emit_z(b, zt):
                # Phase Z: out[o,n] = sum_d m2s[d,o] EV[d,n], + bR on evac.
                # Evacs alternate vector/scalar so neither engine serializes
                # the drain of the final batch.
                m2s, ev, os_ = zt
                for ot in range(KT):
                    osl = slice(ot * 128, (ot + 1) * 128)
                    for h in range(NS):
                        hsl = slice(h * 512, (h + 1) * 512)
                        psz = pp.tile([128, 512], F32, tag="mm")
                        for dp in range(KT // 2):
                            dsl = slice(2 * dp, 2 * dp + 2)
                            nc.tensor.matmul(psz[:], m2s[:, dsl, osl],
                                             ev[:, dsl, hsl],
                                             start=(dp == 0),
                                             stop=(dp == KT // 2 - 1),
                                             perf_mode=DR)
                        if (ot * NS + h) % 2 == 0:
                            nc.vector.tensor_scalar(
                                os_[:, ot, hsl], psz[:], 1.0 / SZ,
                                br[:, ot:ot + 1], mybir.AluOpType.mult,
                                mybir.AluOpType.add)
                        else:
                            nc.scalar.activation(
                                os_[:, ot, hsl], psz[:], AF.Identity,
                                bias=br[:, ot:ot + 1], scale=1.0 / SZ)
                        nc.sync.dma_start(
                            o_d[b, ot * 128:(ot + 1) * 128, hsl],
                            os_[:, ot, hsl])

            with nc.allow_low_precision(
                    reason="fp8 pipeline validated to 5.6e-3 absmax-rel "
                           "against the 2e-2 gate"):
                prev_z = None
                for b in range(BPC):
                    if b == 0:
                        xq, xth, xtl = xq0, xth0, xtl0
                    else:
                        xq = xp.tile([128, KT, N], F8, tag="xq")
                        xth = xp.tile([128, NT, C], F8, tag="xth")
                        xtl = xp.tile([128, NT, C], F8, tag="xtl")
                        nc.sync.dma_start(
                            xq[:], xq_d[b].rearrange("k p n -> p k n"))
                        nc.sync.dma_start(
                            xth[:], xth_d[b].rearrange("t p c -> p t c"))
                        nc.sync.dma_start(
                            xtl[:], xtl_d[b].rearrange("t p c -> p t c"))

                    gen = emit_front(b, xq, xth, xtl)
                    zt = next(gen)
                    if prev_z is not None:
                        emit_z(b - 1, prev_z)
                    for _ in gen:
                        pass
                    prev_z = zt
                emit_z(BPC - 1, prev_z)
    nc.compile()
    return nc


def _in_maps(x, wA, bA, wB, wV, wR, bR):
    import concourse.mybir as mybir
    E4 = mybir.dt.np(mybir.dt.float8e4)

    xr = np.ascontiguousarray(x.reshape(B, C, N), dtype=np.float32)
    xq = xr.astype(E4)                                   # [B, C, N] fp8
    xt = np.swapaxes(xr, 1, 2)                           # [B, N, C]
    xth = np.ascontiguousarray(xt).astype(E4)
    xtl = (xt - xth.astype(np.float32)).astype(E4)       # unscaled residual

    def wq(w):
        hi = np.ascontiguousarray(w.T * WS, dtype=np.float32).astype(E4)
        lo = (w.T * WS - hi.astype(np.float32)).astype(E4)
        return (np.ascontiguousarray(hi.reshape(KT, 128, C)),
                np.ascontiguousarray(lo.reshape(KT, 128, C)))

    wbt = np.ascontiguousarray((wB.T * WS).astype(E4).reshape(KT, 128, C))
    wvt = np.ascontiguousarray((wV.T * WS).astype(E4).reshape(KT, 128, C))
    wath, watl = wq(wA)
    wrth, wrtl = wq(wR)
    qvec = (wR.astype(np.float64) @ bA.astype(np.float64)) * SZ
    qbs = np.ascontiguousarray(
        np.broadcast_to(qvec.reshape(1, C), (128, C)), dtype=np.float32)
    brm = np.ascontiguousarray(bR.reshape(KT, 128).T, dtype=np.float32)
    ones8 = np.ones((128, 2, 128), dtype=E4)
    ones = np.ones((128, 128), dtype=np.float32)

    maps = []
    for i in range(NCORES):
        bsl = slice(i * BPC, (i + 1) * BPC)
        maps.append({
            "xq": np.ascontiguousarray(
                xq[bsl].reshape(BPC, KT, 128, N)),
            "xth": np.ascontiguousarray(
                xth[bsl].reshape(BPC, NT, 128, C)),
            "xtl": np.ascontiguousarray(
                xtl[bsl].reshape(BPC, NT, 128, C)),
            "wbt": wbt, "wvt": wvt, "wath": wath, "watl": watl,
            "wrth": wrth, "wrtl": wrtl, "qbs": qbs, "br": brm,
            "ones8": ones8, "ones": ones,
        })
    return maps


def kernel(x, wA, bA, wB, bB, wV, bV, wR, bR):
    from concourse.bass_utils import run_bass_kernel_spmd
    if "nc" not in _CACHE:
        _CACHE["nc"] = _build_nc()
    nc = _CACHE["nc"]
    maps = _in_maps(x, wA, bA, wB, wV, wR, bR)
    res = run_bass_kernel_spmd(nc, maps, list(range(NCORES)))
    out = np.concatenate(
        [res.results[i]["o"].astype(np.float32) for i in range(NCORES)],
        axis=0)
    return out.reshape(B, C, H, W)
